# revision 14
# baseline (speedup 1.0000x reference)
"""Trainium2 Bass kernel for nn_Allocator2 (dense_cnn), 8 NeuronCores.

Pure data parallelism: batch 64 -> 8 samples per core, weights replicated.

v2: restructured from the 19-pass baseline to 13 array-passes per column:
  - dilated conv outputs reordered hh-major and split into P_A (hh 0-4,
    125 rows) / P_B (hh 4-6, 75 rows) so F1 becomes 6 offset-generations
    of TWO CONCURRENT col-tiled matmuls (tile_position (0,0) M=64 and
    (0,64) M=32) instead of 9 serial passes.
  - F1 output is written h-major (psum partition h*16+o) and baked
    directly from PSUM into dw-parity tiles a1b_A (p in {0,1} x h 0-2,
    rows 0-47 / 64-111) and a1b_B (p x h 2-5, rows 0-63 / 64-127);
    F2 is 3 offset-generations of two concurrent matmuls (M=16 @ (0,0),
    M=24 @ (0,32)).
  - F3 keeps the 3-shift bake (a2b rows k*3+p) but the bake DMA runs on
    the hardware DGE rings (sync/scalar queues), as does the S-window
    build; NO software (gpsimd) DMA queues anywhere.
  - all activations/bakes are engine ops at 32-aligned partition bases.

All matmuls bf16 operands, fp32 PSUM accumulation.
"""

import numpy as np
import ml_dtypes

BF16 = ml_dtypes.bfloat16

B = 64            # global batch
NCORES = 8
BS = B // NCORES  # 8 samples per core
ND = 25
L = 8192          # concat length (4096 + 4096)
LX = 4096
LC = L - ND       # 8167 dilated output length
T1 = LC - 5       # 8162 F1 output length
T2 = T1 - 5       # 8157
T3 = T2 - 5       # 8152
NT = 512          # matmul free-dim tile


def _bd(blocks):
    """block-diagonal stack of 2D arrays"""
    rs = sum(b.shape[0] for b in blocks)
    cs = sum(b.shape[1] for b in blocks)
    out = np.zeros((rs, cs), np.float32)
    r = c = 0
    for b in blocks:
        out[r:r + b.shape[0], c:c + b.shape[1]] = b
        r += b.shape[0]
        c += b.shape[1]
    return out


def build_weights(inp):
    """Host-side weight prep. Returns dict of np arrays (bf16 weights,
    fp32 biases) shared by all cores."""
    w = {}
    f32 = np.float32

    # ---- head: block-diagonal over BS samples, lhsT layout [K, M] ----
    def head_lhsT(wmat):  # wmat [Co, Ci] -> lhsT [Ci, Co] per sample
        return _bd([wmat.T.astype(f32)] * BS)

    w['hT1'] = head_lhsT(inp['wT1'])   # [72, 48]
    w['hT2'] = head_lhsT(inp['wT2'])   # [48, 32]
    w['hT3'] = head_lhsT(inp['wT3'])   # [32, 16]
    w['hR1'] = head_lhsT(inp['wR1'])   # [24, 16]
    w['hR2'] = head_lhsT(inp['wR2'])   # [16, 16]
    for nm in ('bT1', 'bT2', 'bT3', 'bR1', 'bR2'):
        w['h' + nm] = np.tile(inp[nm].astype(f32), BS)[:, None]  # [BS*Co, 1]

    # ---- dilated: hh-major M layout; S rows r=(c*26+sh) ----
    # output m=(hh=o)*25 + (ci=i); dA covers hh 0..4 (125), dB hh 4..6 (75)
    wM = inp['wM'].astype(f32)  # [25, 7, 2, 2]
    dil = np.zeros((52, 7, 25), f32)  # [K, hh, ci]
    for i in range(ND):
        for o in range(7):
            for c in range(2):
                dil[c * 26 + 0, o, i] = wM[i, o, c, 0]
                dil[c * 26 + (i + 1), o, i] = wM[i, o, c, 1]
    w['dA'] = dil[:, 0:5].reshape(52, 125)
    w['dB'] = dil[:, 2:7].reshape(52, 125)
    bM = inp['bM'].astype(f32)  # [25, 7] -> [hh*25+ci]
    w['bA'] = bM.T[0:5].reshape(125, 1).astype(f32)
    w['bB'] = bM.T[2:7].reshape(125, 1).astype(f32)

    # ---- F1: per dw, two lhsT blocks ----
    # W1a[dw] [125, 64]: row hh*25+ci (hh 0-4), col h*16+o (h 0-3)
    # W1b[dw] [125, 64]: row (hh-2)*25+ci (hh 2-6), col (h-2)*16+o (h 2-5)
    wF1 = inp['wF1'].astype(f32)  # [16, 25, 2, 6]  [o, ci, dh, dw]
    f1a = np.zeros((6, 125, 64), f32)
    f1b = np.zeros((6, 125, 64), f32)
    for dw in range(6):
        for ci in range(25):
            for hh in range(7):
                for o in range(16):
                    for h in range(6):
                        dh = hh - h
                        if not (0 <= dh <= 1):
                            continue
                        if hh <= 4 and h <= 3:
                            f1a[dw, hh * 25 + ci, h * 16 + o] = wF1[o, ci, dh, dw]
                        if hh >= 2 and h >= 2:
                            f1b[dw, (hh - 2) * 25 + ci,
                                (h - 2) * 16 + o] = wF1[o, ci, dh, dw]
    w['F1A'] = f1a
    w['F1B'] = f1b
    # a1 rows r = h*16 + o (h 0-5): bias value bF1[o]
    w['bF1'] = np.tile(inp['bF1'].astype(f32), 6)[:, None]  # [96, 1]
    w['bF1B'] = np.tile(inp['bF1'].astype(f32), 4)[:, None]  # [64, 1] h 2-5

    # ---- F2: baked-parity lhsT blocks ----
    # a1b_A [112]: row p*64 + (h*16+ci) for h 0-2 (rows 48-63 zero junk)
    # a1b_B [128]: row p*64 + ((h-2)*16+ci) for h 2-5
    # W2a[g] [112, 16]: col h'*8+o' (h' 0-1); W2b[g] [128, 24]: col (h'-2)*8+o'
    wF2 = inp['wF2'].astype(f32)  # [8, 16, 2, 6]  [o', ci, dh, dw]
    f2a = np.zeros((3, 112, 16), f32)
    f2b = np.zeros((3, 128, 24), f32)
    for g in range(3):
        for p in range(2):
            dw = 2 * g + p
            for ci in range(16):
                for h in range(6):
                    for o2 in range(8):
                        for h2 in range(5):
                            dh = h - h2
                            if not (0 <= dh <= 1):
                                continue
                            if h <= 2 and h2 <= 1:
                                f2a[g, p * 64 + h * 16 + ci,
                                    h2 * 8 + o2] = wF2[o2, ci, dh, dw]
                            if h >= 2 and h2 >= 2:
                                f2b[g, p * 64 + (h - 2) * 16 + ci,
                                    (h2 - 2) * 8 + o2] = wF2[o2, ci, dh, dw]
    w['F2A'] = f2a
    w['F2B'] = f2b
    # a2 rows r = h'*8 + o' (h' 0-4): bias bF2[o']; stored padded [64]:
    # rows 0-15 = r 0-15 (h' 0-1), rows 32-55 = r 16-39 (h' 2-4)
    b2 = np.tile(inp['bF2'].astype(f32), 5)  # [40] value per r
    b2p = np.zeros((64, 1), f32)
    b2p[0:16, 0] = b2[0:16]
    b2p[32:56, 0] = b2[16:40]
    w['bF2'] = b2p

    # ---- F3 baked x3: lhsT[g] [120, 4]; row (k*3+p), k = h'*8+o' ----
    wF3 = inp['wF3'].astype(f32)  # [1, 8, 2, 6]
    f3 = np.zeros((2, 120, 4), f32)
    for g in range(2):
        for p in range(3):
            dw = g * 3 + p
            for o2 in range(8):
                for h2 in range(5):
                    for h3 in range(4):
                        dh = h2 - h3
                        if 0 <= dh <= 1:
                            f3[g, (h2 * 8 + o2) * 3 + p, h3] = wF3[0, o2, dh, dw]
    w['F3'] = f3
    w['thr'] = np.full((4, 1), -inp['bF3'][0], f32)  # out = (psum > thr)

    # bf16-ify matmul weights
    for k in ('hT1', 'hT2', 'hT3', 'hR1', 'hR2', 'dA', 'dB',
              'F1A', 'F1B', 'F2A', 'F2B', 'F3'):
        w[k] = w[k].astype(BF16)
    return w


def emulate_core(w, x_core, y_core):
    """Numpy emulation of exactly what the Bass kernel computes for one
    core. x_core [72, 4096] bf16, y_core [24, 4096] bf16. Returns
    [BS, 4, T3] f32 in {0,1}."""
    f32 = np.float32

    def mm(lhsT, rhs):  # bf16 operands, f32 accumulate
        return lhsT.astype(f32).T @ rhs.astype(f32)

    relu = lambda a: np.maximum(a, 0)
    sig = lambda a: 1.0 / (1.0 + np.exp(-a))

    a = relu(mm(w['hT1'], x_core) + w['hbT1']).astype(BF16)
    a = relu(mm(w['hT2'], a) + w['hbT2']).astype(BF16)
    t3 = (mm(w['hT3'], a) + w['hbT3']).astype(BF16)          # [16, 4096]
    b_ = relu(mm(w['hR1'], y_core) + w['hbR1']).astype(BF16)
    b_ = relu(mm(w['hR2'], b_) + w['hbR2']).astype(BF16)     # [16, 4096]
    out2 = np.concatenate([t3, b_], axis=1)                  # [16, 8192] bf16

    res = np.zeros((BS, 4, T3), f32)
    for s in range(BS):
        o2 = out2[s * 2:s * 2 + 2]                           # [2, 8192]
        S = np.zeros((52, LC), BF16)
        for c in range(2):
            for sh in range(26):
                S[c * 26 + sh] = o2[c, sh:sh + LC]
        PA = relu(mm(w['dA'], S) + w['bA']).astype(BF16)     # [125, LC] hh 0-4
        PB = relu(mm(w['dB'], S) + w['bB']).astype(BF16)     # [125, LC] hh 2-6
        # F1: 6 offsets, two blocks -> ps1 [128, T1]
        z1 = np.zeros((128, T1), f32)
        for dw in range(6):
            z1[0:64] += mm(w['F1A'][dw], PA[:, dw:dw + T1])    # h 0-3
            z1[64:128] += mm(w['F1B'][dw], PB[:, dw:dw + T1])  # h 2-5
        a1A = sig(z1[0:48] + w['bF1'][0:48]).astype(BF16)    # a1 rows h 0-2
        a1B = sig(z1[64:128] + w['bF1B']).astype(BF16)       # a1 rows h 2-5
        # baked tiles (junk rows zero)
        a1bA = np.zeros((112, T1), BF16)
        a1bB = np.zeros((128, T1), BF16)
        for p in range(2):
            a1bA[p * 64:p * 64 + 48, :T1 - p] = a1A[:, p:T1]
            a1bB[p * 64:p * 64 + 64, :T1 - p] = a1B[:, p:T1]
        z2 = np.zeros((40, T2), f32)
        for g in range(3):
            z2[0:16] += mm(w['F2A'][g], a1bA[:, 2 * g:2 * g + T2])
            z2[16:40] += mm(w['F2B'][g], a1bB[:, 2 * g:2 * g + T2])
        a2 = sig(z2 + np.concatenate([w['bF2'][0:16], w['bF2'][32:56]])
                 ).astype(BF16)                              # [40, T2] row h'*8+o'
        a2b = np.zeros((120, T3 + 3), BF16)
        for k in range(40):
            for p in range(3):
                a2b[k * 3 + p] = a2[k, p:p + T3 + 3]
        z3 = mm(w['F3'][0], a2b[:, :T3]) + mm(w['F3'][1], a2b[:, 3:3 + T3])
        res[s] = (z3 > w['thr']).astype(f32)                 # [4, T3]
    return res


def _shard_inputs(inputs):
    """Build per-core in_maps (host-side prep + shard)."""
    w = build_weights(inputs)
    in_maps = []
    for c in range(NCORES):
        m = dict(w)
        xs = inputs['x'][c * BS:(c + 1) * BS]  # [8, 9, 4096]
        ys = inputs['y'][c * BS:(c + 1) * BS]
        m['x'] = np.ascontiguousarray(xs.reshape(BS * 9, LX)).astype(BF16)
        m['y'] = np.ascontiguousarray(ys.reshape(BS * 3, LX)).astype(BF16)
        in_maps.append(m)
    return in_maps


# ---------------------------------------------------------------------------
# Bass program
# ---------------------------------------------------------------------------

def _split_excess_waits(bir, maxw=1):
    """The walrus build in this container refuses instructions carrying
    more than ~1 semaphore wait ("Too many sync wait commands").  Tile
    attaches multi-waits freely.  Splitting is semantics-preserving: move
    excess waits onto injected NoOps on the same engine immediately
    before the instruction (engines execute their instruction stream in
    order, so wait-all is preserved)."""
    for fn in bir['functions']:
        for bb in fn['blocks']:
            out = []
            for inst in bb['instructions']:
                si = inst.get('sync_info')
                waits = (si or {}).get('on_wait') or []
                if len(waits) > maxw:
                    extra, keep = waits[:-maxw], waits[-maxw:]
                    for i in range(0, len(extra), maxw):
                        out.append({
                            "debug": inst.get("debug", 0),
                            "engine": inst["engine"], "ins": [],
                            "name": f"{inst['name']}-wsplit{i}",
                            "opcode": "NoOp", "outs": [],
                            "sync_info": {"on_update": [],
                                          "on_wait": extra[i:i + maxw]}})
                    si['on_wait'] = keep
                out.append(inst)
            bb['instructions'] = out
    return bir


def _patch_serialization(nc):
    import orjson
    bir = _split_excess_waits(nc.to_json())
    patched = orjson.dumps(bir)
    nc.to_json_bytes = lambda: patched
    return nc


def build_bass():
    import bass_rust
    import concourse.bass as bass
    import concourse.mybir as mybir
    from concourse.tile import TileContext

    dt = mybir.dt
    AF = mybir.ActivationFunctionType
    ALU = mybir.AluOpType

    nc = bass.Bass()

    p = {}
    p['x'] = nc.declare_dram_parameter('x', [BS * 9, LX], dt.bfloat16, False)
    p['y'] = nc.declare_dram_parameter('y', [BS * 3, LX], dt.bfloat16, False)
    for nm, sh in [('hT1', [BS * 9, BS * 6]), ('hT2', [BS * 6, BS * 4]),
                   ('hT3', [BS * 4, BS * 2]),
                   ('hR1', [BS * 3, BS * 2]), ('hR2', [BS * 2, BS * 2]),
                   ('dA', [52, 125]), ('dB', [52, 125]),
                   ('F1A', [6, 125, 64]), ('F1B', [6, 125, 64]),
                   ('F2A', [3, 112, 16]), ('F2B', [3, 128, 24]),
                   ('F3', [2, 120, 4])]:
        p[nm] = nc.declare_dram_parameter(nm, sh, dt.bfloat16, False)
    for nm, sh in [('hbT1', [BS * 6, 1]), ('hbT2', [BS * 4, 1]),
                   ('hbT3', [BS * 2, 1]),
                   ('hbR1', [BS * 2, 1]), ('hbR2', [BS * 2, 1]),
                   ('bA', [125, 1]), ('bB', [125, 1]),
                   ('bF1', [96, 1]), ('bF1B', [64, 1]), ('bF2', [64, 1]),
                   ('thr', [4, 1])]:
        p[nm] = nc.declare_dram_parameter(nm, sh, dt.float32, False)
    out_d = nc.declare_dram_parameter('out', [BS * 4, T3], dt.bfloat16, True)

    def ceil_div(a, b):
        return -(-a // b)

    NTILES = ceil_div(LC, NT)   # 16 column tiles of 512

    with TileContext(nc) as tc:
        with tc.tile_pool(name="wpool", bufs=1) as wp, \
             tc.tile_pool(name="big", bufs=1) as bp, \
             tc.tile_pool(name="head", bufs=1) as hp, \
             tc.tile_pool(name="psum", bufs=8, space="PSUM") as pp:

            W = {}
            for nm in ('hT1', 'hT2', 'hT3', 'hR1', 'hR2', 'dA', 'dB',
                       'hbT1', 'hbT2', 'hbT3', 'hbR1', 'hbR2',
                       'bA', 'bB', 'bF1', 'bF1B', 'bF2', 'thr'):
                t = wp.tile(list(p[nm].shape), p[nm].dtype, name=f"w_{nm}")
                nc.sync.dma_start(out=t[...], in_=p[nm][...])
                W[nm] = t
            for nm in ('F1A', 'F1B', 'F2A', 'F2B', 'F3'):
                n_sl, kk, mm_ = p[nm].shape
                W[nm] = []
                for i_sl in range(n_sl):
                    t = wp.tile([kk, mm_], p[nm].dtype, name=f"w_{nm}{i_sl}")
                    nc.sync.dma_start(out=t[...], in_=p[nm][i_sl])
                    W[nm].append(t)

            # persistent big tiles (shared across samples)
            o2t = bp.tile([16, L], dt.bfloat16, name="o2t")
            PA = bp.tile([125, LC], dt.bfloat16, name="PA")
            PB = bp.tile([125, LC], dt.bfloat16, name="PB")
            a1bA = bp.tile([112, T1], dt.bfloat16, name="a1bA")
            a1bB = bp.tile([128, T1], dt.bfloat16, name="a1bB")
            a2t = bp.tile([64, T2], dt.bfloat16, name="a2t")

            # zero junk rows once (F2A matmul reads them with zero weights)
            nc.vector.memset(a1bA[32:64, :], 0.0)

            # ---------------- head: all samples stacked ----------------
            xt = hp.tile([BS * 9, LX], dt.bfloat16, name="xt")
            yt = hp.tile([BS * 3, LX], dt.bfloat16, name="yt")
            nc.sync.dma_start(out=xt[...], in_=p['x'][...])
            nc.sync.dma_start(out=yt[...], in_=p['y'][...])

            def head_col(w_nm, b_nm, rows_in, rows_out, src, src_sl,
                         dst, dst_sl, eng):
                ps = pp.tile([128, NT], dt.float32, tag="ps")
                nc.tensor.matmul(ps[:rows_out], W[w_nm][...],
                                 src[:rows_in, src_sl], start=True, stop=True)
                if eng == 'scalar':
                    nc.scalar.activation(dst[:rows_out, dst_sl],
                                         ps[:rows_out], AF.Relu,
                                         bias=W[b_nm][...])
                elif eng == 'vrelu':
                    nc.vector.tensor_scalar(dst[:rows_out, dst_sl],
                                            ps[:rows_out], W[b_nm][...],
                                            0.0, ALU.add, ALU.max)
                else:  # plain add (T3)
                    nc.vector.tensor_scalar(dst[:rows_out, dst_sl],
                                            ps[:rows_out],
                                            W[b_nm][...], None, ALU.add)

            S_tiles = {}

            def build_S_part(s, h0, h1):
                St = S_tiles[s]
                for c in range(2):
                    win = o2t[s * 2 + c:s * 2 + c + 1, h0:h1].copy()
                    win.ap = bass_rust.VecI64Pair(
                        [[L, 1], [1, 26], [1, h1 - h0]])
                    q = nc.sync if c == 0 else nc.scalar
                    q.dma_start(
                        out=St[c * 26:(c + 1) * 26, h0:h1], in_=win)

            def build_S(s):
                St = bp.tile([52, LC], dt.bfloat16, tag="S", bufs=2,
                             name="St")
                S_tiles[s] = St
                build_S_part(s, 0, 4071)
                build_S_part(s, 4071, LC)

            for s in (0, 1):
                S_tiles[s] = bp.tile([52, LC], dt.bfloat16, tag="S", bufs=2,
                                     name="St")
            # column-pipelined head: both paths complete per column tile,
            # S windows for samples 0/1 stream out on the DGE rings behind
            pT = [0, 0]
            pR = [4071, 4071]
            for j in range(LX // NT):
                sl = slice(j * NT, (j + 1) * NT)
                slR = slice(LX + j * NT, LX + (j + 1) * NT)
                fl = slice(0, NT)
                a1h = hp.tile([BS * 6, NT], dt.bfloat16, tag="htmp", bufs=4,
                              name="a1h")
                a2h = hp.tile([BS * 6, NT], dt.bfloat16, tag="htmp", bufs=4,
                              name="a2h")
                b1h = hp.tile([BS * 6, NT], dt.bfloat16, tag="htmp", bufs=4,
                              name="b1h")
                head_col('hT1', 'hbT1', BS * 9, BS * 6, xt, sl, a1h, fl,
                         'vrelu')
                head_col('hT2', 'hbT2', BS * 6, BS * 4, a1h, fl, a2h, fl,
                         'scalar')
                head_col('hT3', 'hbT3', BS * 4, BS * 2, a2h, fl, o2t, sl,
                         'add')
                head_col('hR1', 'hbR1', BS * 3, BS * 2, yt, sl, b1h, fl,
                         'scalar')
                head_col('hR2', 'hbR2', BS * 2, BS * 2, b1h, fl, o2t, slR,
                         'vrelu')
                availT = min((j + 1) * NT - 25, 4071)
                availR = min(LX + (j + 1) * NT - 25, LC)
                for s in (0, 1):
                    if availT - pT[s] >= 1024 or (j == LX // NT - 1
                                                  and availT > pT[s]):
                        build_S_part(s, pT[s], availT)
                        pT[s] = availT
                    if availR - pR[s] >= 1024 or (j == LX // NT - 1
                                                  and availR > pR[s]):
                        build_S_part(s, pR[s], availR)
                        pR[s] = availR
            for s in (0, 1):
                if pT[s] < 4071:
                    build_S_part(s, pT[s], 4071)
                if pR[s] < LC:
                    build_S_part(s, pR[s], LC)

            # ---------------- per-tile stage emitters ------------------
            def emit_dil(s, j):
                St = S_tiles[s]
                t0 = j * NT
                nt = min(NT, LC - t0)
                psA = pp.tile([128, NT], dt.float32, tag="ps")
                psB = pp.tile([128, NT], dt.float32, tag="ps")
                nc.tensor.matmul(psA[0:125, :nt], W['dA'][...],
                                 St[:, t0:t0 + nt], start=True, stop=True)
                nc.tensor.matmul(psB[0:125, :nt], W['dB'][...],
                                 St[:, t0:t0 + nt], start=True, stop=True)
                nc.vector.tensor_scalar(PA[:, t0:t0 + nt], psA[0:125, :nt],
                                        W['bA'][...], 0.0, ALU.add, ALU.max)
                if j % 2 == 0:
                    nc.scalar.activation(PB[:, t0:t0 + nt], psB[0:125, :nt],
                                         AF.Relu, bias=W['bB'][...])
                else:
                    nc.vector.tensor_scalar(PB[:, t0:t0 + nt], psB[0:125, :nt],
                                            W['bB'][...], 0.0, ALU.add, ALU.max)

            def emit_f1(s, j):
                t0 = j * NT
                nt = min(NT, T1 - t0)
                ps1 = pp.tile([128, NT], dt.float32, tag="ps")
                for dw in range(6):
                    nc.tensor.matmul(ps1[0:64, :nt], W['F1A'][dw],
                                     PA[:, t0 + dw:t0 + dw + nt],
                                     start=(dw == 0), stop=(dw == 5),
                                     tile_position=(0, 0))
                    nc.tensor.matmul(ps1[64:128, :nt], W['F1B'][dw],
                                     PB[:, t0 + dw:t0 + dw + nt],
                                     start=(dw == 0), stop=(dw == 5),
                                     tile_position=(0, 64))
                nc.scalar.activation(a1bA[0:48, t0:t0 + nt], ps1[0:48, :nt],
                                     AF.Sigmoid, bias=W['bF1'][0:48])
                nc.scalar.activation(a1bB[0:64, t0:t0 + nt], ps1[64:128, :nt],
                                     AF.Sigmoid, bias=W['bF1B'][...])
                ceng = nc.vector if (j < 2 or j % 2 == 0) else nc.gpsimd
                if j == 0:
                    ceng.tensor_copy(a1bA[64:112, 0:nt - 1],
                                     a1bA[0:48, 1:nt])
                    ceng.tensor_copy(a1bB[64:128, 0:nt - 1],
                                     a1bB[0:64, 1:nt])
                else:
                    ceng.tensor_copy(a1bA[64:112, t0 - 1:t0 + nt - 1],
                                     a1bA[0:48, t0:t0 + nt])
                    ceng.tensor_copy(a1bB[64:128, t0 - 1:t0 + nt - 1],
                                     a1bB[0:64, t0:t0 + nt])

            def emit_f2(s, j):
                t0 = j * NT
                nt = min(NT, T2 - t0)
                ps2 = pp.tile([128, NT], dt.float32, tag="ps")
                for g in range(3):
                    nc.tensor.matmul(ps2[0:16, :nt], W['F2A'][g],
                                     a1bA[:, t0 + 2 * g:t0 + 2 * g + nt],
                                     start=(g == 0), stop=(g == 2),
                                     tile_position=(0, 0))
                    nc.tensor.matmul(ps2[32:56, :nt], W['F2B'][g],
                                     a1bB[:, t0 + 2 * g:t0 + 2 * g + nt],
                                     start=(g == 0), stop=(g == 2),
                                     tile_position=(0, 32))
                nc.scalar.activation(a2t[0:56, t0:t0 + nt], ps2[0:56, :nt],
                                     AF.Sigmoid, bias=W['bF2'][0:56])
                if j % 2 == 1 or t0 + nt >= T2:
                    # bake the finished 1024 block (sync HW DGE):
                    # a2b[k*3+p, c] = a2[k, c+p]
                    a2b = a2b_tiles[s]
                    b0 = max((j // 2) * 2 * NT - 2, 0)
                    b1 = min(t0 + nt - 2, T3 + 3)
                    for (r0, r1, d0, q) in ((0, 16, 0, nc.sync),
                                            (32, 56, 48, nc.scalar)):
                        win = a2t[r0:r1, b0:b1].copy()
                        win.ap = bass_rust.VecI64Pair(
                            [[T2, r1 - r0], [1, 3], [1, b1 - b0]])
                        q.dma_start(
                            out=a2b[d0:d0 + (r1 - r0) * 3, b0:b1], in_=win)

            def emit_f3(s, j, otb):
                a2b = a2b_tiles[s]
                t0 = j * NT
                nt = min(NT, T3 - t0)
                ps3 = pp.tile([128, NT], dt.float32, tag="ps")
                for g in range(2):
                    nc.tensor.matmul(ps3[0:4, :nt], W['F3'][g],
                                     a2b[:, t0 + 3 * g:t0 + 3 * g + nt],
                                     start=(g == 0), stop=(g == 1))
                nc.vector.tensor_scalar(otb[:, (j % 2) * NT:(j % 2) * NT + nt],
                                        ps3[0:4, :nt],
                                        W['thr'][...], None, ALU.is_gt)
                if j % 2 == 1 or t0 + nt >= T3:
                    c0 = (j // 2) * 2 * NT
                    span = t0 + nt - c0
                    nc.sync.dma_start(out=out_d[s * 4:(s + 1) * 4,
                                                c0:c0 + span],
                                      in_=otb[:, 0:span])

            a2b_tiles = {}

            # ---------------- two-phase pipelined emission -------------
            # iteration s: [dil(s) ~ F1(s)] then [F2(s) ~ F3(s-1)].
            # Pairing a matmul-heavy stage with each drain-heavy stage
            # keeps TensorE fed while scalar/vector bake chains chase.
            n3t = ceil_div(T3, NT)
            for s in range(BS):
                a2b_s = bp.tile([120, T3 + 3], dt.bfloat16,
                                tag="a2b", bufs=2, name="a2b")
                a2b_tiles[s] = a2b_s
                for j in range(NTILES + 1):
                    if j < NTILES:
                        emit_dil(s, j)
                    if 1 <= j:
                        emit_f1(s, j - 1)
                if s + 2 < BS:
                    build_S(s + 2)
                otb = None
                for j in range(NTILES):
                    emit_f2(s, j)
                    if s > 0 and j < n3t:
                        if j % 2 == 0:
                            otb = bp.tile([4, 2 * NT], dt.bfloat16,
                                          tag="ot", bufs=2)
                        emit_f3(s - 1, j, otb)
            s = BS - 1
            for j in range(n3t):
                if j % 2 == 0:
                    otb = bp.tile([4, 2 * NT], dt.bfloat16, tag="ot", bufs=2)
                emit_f3(s, j, otb)

    return _patch_serialization(nc)


def kernel(**inputs):
    inputs = {k: np.asarray(v) for k, v in inputs.items()}
    in_maps = _shard_inputs(inputs)
    nc = build_bass()
    from concourse.bass_utils import run_bass_kernel_spmd
    res = run_bass_kernel_spmd(nc, in_maps, core_ids=list(range(NCORES)))
    outs = [res.results[i]['out'].reshape(BS, 4, T3) for i in range(NCORES)]
    full = np.concatenate(outs, axis=0)[:, None]  # [64, 1, 4, T3]
    return full.astype(np.float32)
